# revision 1
# baseline (speedup 1.0000x reference)
"""BigBird encoder + vocab projection on 8 Trainium2 NeuronCores.

Sequence-sharded transformer (core c owns rows [256c, 256c+256) = 4 query
blocks), per-layer AllGather of K^T and V, vocab-sharded final projection.
Activations live transposed (xT [768part, 256free]) so every matmul consumes
weights as stored.  BigBird sparsity (window + global + random + dedup +
key_mask) is folded into a per-core 0/1 mask multiplied into exp(scores) —
mathematically identical to the reference's gather+softmax.  bf16 matmuls,
fp32 accumulate/residual.
"""
import os, sys
os.environ.setdefault("JAX_PLATFORMS", "")
import numpy as np
import ml_dtypes

sys.path.insert(0, "/opt/trn_rl_repo")

import concourse.bass as bass
import concourse.tile as tile
from concourse import bacc, mybir
from concourse.bass_utils import run_bass_kernel_spmd

BF16 = mybir.dt.bfloat16
F32 = mybir.dt.float32
AF = mybir.ActivationFunctionType
MUL = mybir.AluOpType.mult
ADD = mybir.AluOpType.add
SUB = mybir.AluOpType.subtract

B, S, D, F, V = 1, 2048, 768, 3072, 50358
L, H, HD, BS, NB, R = 12, 12, 64, 64, 32, 3
NC = 8
SQ = S // NC                # 256
DC = D // 128               # 6
FC = F // 128               # 24
KC = S // 128               # 16
VSH = 6400                  # padded vocab shard (50 x 128)
VN_E = HD + 1               # 65 cols per head in V-normal (ones col for rowsum)
VN_FLAT = 128 * 2 * H * VN_E    # 199680
KT_FLAT = 128 * DC * SQ         # 196608
AG_FLAT = VN_FLAT + KT_FLAT     # 396288
XF_FLAT = 128 * DC * SQ

_nc_cache = {}


def _block_map(nb, r, seed=0):
    rng = np.random.default_rng(seed)
    idx = np.zeros((nb, 5 + r), np.int32)
    for i in range(nb):
        lst = [0, nb - 1, max(i - 1, 0), i, min(i + 1, nb - 1)]
        cand = np.setdiff1d(np.arange(nb), np.array(lst))
        lst += list(rng.choice(cand, r, replace=False))
        for j, b in enumerate(lst):
            idx[i, j] = int(b)
    return idx


def build():
    nc = bacc.Bacc("TRN2", target_bir_lowering=False, debug=False, num_devices=NC)
    ET = nc.dram_tensor("e_t", [128, DC, SQ], F32, kind="ExternalInput")
    WQ = nc.dram_tensor("wq", [L, 128, DC, D], BF16, kind="ExternalInput")
    WK = nc.dram_tensor("wk", [L, 128, DC, D], BF16, kind="ExternalInput")
    WV = nc.dram_tensor("wv", [L, 128, DC, D], BF16, kind="ExternalInput")
    WO = nc.dram_tensor("wo", [L, 128, DC, D], BF16, kind="ExternalInput")
    W1 = nc.dram_tensor("w1", [L, 128, DC, F], BF16, kind="ExternalInput")
    W2 = nc.dram_tensor("w2", [L, 128, FC, D], BF16, kind="ExternalInput")
    BQ = nc.dram_tensor("bq_t", [L, 128, DC], F32, kind="ExternalInput")  # x0.125
    BK = nc.dram_tensor("bk_t", [L, 128, DC], F32, kind="ExternalInput")
    B1 = nc.dram_tensor("b1_t", [L, 128, FC], F32, kind="ExternalInput")
    BROW = nc.dram_tensor("brow", [L, 3, D], BF16, kind="ExternalInput")  # bv,bo,b2
    LNS = nc.dram_tensor("ln_s", [128, 2 * L + 1, DC], F32, kind="ExternalInput")
    LNB = nc.dram_tensor("ln_b", [128, 2 * L + 1, DC], F32, kind="ExternalInput")
    M01 = nc.dram_tensor("m01t", [128, KC, 4], BF16, kind="ExternalInput")
    FCW = nc.dram_tensor("fcw", [128, DC, VSH], BF16, kind="ExternalInput")
    FCB = nc.dram_tensor("fcb", [1, VSH], BF16, kind="ExternalInput")
    OUT = nc.dram_tensor("out_t", [VSH // 128, 128, S], F32, kind="ExternalOutput")

    with tile.TileContext(nc) as tc:
        with tc.tile_pool(name="dram", bufs=1, space="DRAM") as dram, \
             tc.tile_pool(name="res", bufs=1) as res, \
             tc.tile_pool(name="const", bufs=1) as const:
            bnc_ins = [dram.tile([AG_FLAT], BF16, tag=f"bin{l}", name=f"bin{l}")
                       for l in range(L)]
            bnc_outs = [dram.tile([NC * AG_FLAT], BF16, addr_space="Shared",
                                  tag=f"bout{l}", name=f"bout{l}")
                        for l in range(L)]
            bncx_in = dram.tile([XF_FLAT], BF16)
            bncx_out = dram.tile([NC * XF_FLAT], BF16, addr_space="Shared")

            xT = res.tile([128, DC, SQ], F32)
            nc.sync.dma_start(xT[:], ET[:])
            m01 = const.tile([128, KC, 4], BF16)
            nc.sync.dma_start(m01[:], M01[:])
            lns = const.tile([128, 2 * L + 1, DC], F32)
            lnb = const.tile([128, 2 * L + 1, DC], F32)
            nc.sync.dma_start(lns[:], LNS[:])
            nc.sync.dma_start(lnb[:], LNB[:])
            ones_b = const.tile([128, 1], BF16)
            nc.vector.memset(ones_b[:], 1.0)
            ones_fr = const.tile([1, 128], F32)
            nc.vector.memset(ones_fr[:], 1.0)
            onesrow = const.tile([1, 512], BF16)
            nc.vector.memset(onesrow[:], 1.0)
            eps = const.tile([1, 1], F32)
            nc.vector.memset(eps[:], 1e-12)

            with tc.tile_pool(name="wp", bufs=1) as wp, \
                 tc.tile_pool(name="act", bufs=2) as act, \
                 tc.tile_pool(name="ag", bufs=1) as ag, \
                 tc.tile_pool(name="ps", bufs=2, space="PSUM") as ps, \
                 tc.tile_pool(name="pssc", bufs=2, space="PSUM") as pssc, \
                 tc.tile_pool(name="ps1", bufs=2, space="PSUM") as ps1, \
                 tc.tile_pool(name="sm", bufs=2) as sm:

                def layer_norm(li):
                    xbf = act.tile([128, DC, SQ], BF16, tag="xbf", name="lnxbf")
                    nc.vector.tensor_copy(xbf[:], xT[:])
                    sq = act.tile([128, DC, SQ], BF16, tag="sq", bufs=1)
                    nc.scalar.activation(sq[:], xbf[:], AF.Square)
                    sum_ps = pssc.tile([1, SQ], F32, tag="sc", name="lnsum")
                    ssq_ps = pssc.tile([1, SQ], F32, tag="sc", name="lnssq")
                    for i in range(DC):
                        nc.tensor.matmul(sum_ps[:], ones_b[:], xbf[:, i, :],
                                         start=(i == 0), stop=(i == DC - 1))
                    for i in range(DC):
                        nc.tensor.matmul(ssq_ps[:], ones_b[:], sq[:, i, :],
                                         start=(i == 0), stop=(i == DC - 1))
                    nmean = sm.tile([1, SQ], F32, tag="nmean")
                    ms = sm.tile([1, SQ], F32, tag="ms")
                    nc.vector.tensor_scalar_mul(nmean[:], sum_ps[:], -1.0 / D)
                    nc.vector.tensor_scalar_mul(ms[:], ssq_ps[:], 1.0 / D)
                    ab = sm.tile([1, 2 * SQ], F32, tag="ab")
                    # var -> ab[0:SQ]
                    nc.vector.tensor_tensor(ab[:, 0:SQ], nmean[:], nmean[:], op=MUL)
                    nc.vector.tensor_tensor(ab[:, 0:SQ], ms[:], ab[:, 0:SQ], op=SUB)
                    nc.scalar.activation(ab[:, 0:SQ], ab[:, 0:SQ], AF.Sqrt,
                                         bias=eps[:])
                    nc.vector.reciprocal(ab[:, 0:SQ], ab[:, 0:SQ])   # rstd
                    nc.vector.tensor_tensor(ab[:, SQ:], nmean[:], ab[:, 0:SQ],
                                            op=MUL)                  # -mean*rstd
                    bc = pssc.tile([128, 2 * SQ], F32, tag="sc", name="lnbc")
                    nc.tensor.matmul(bc[:], ones_fr[:], ab[:], start=True, stop=True)
                    nc.vector.tensor_tensor(
                        xT[:], xT[:],
                        bc[:, None, 0:SQ].to_broadcast([128, DC, SQ]), op=MUL)
                    nc.vector.tensor_tensor(
                        xT[:], xT[:],
                        bc[:, None, SQ:2 * SQ].to_broadcast([128, DC, SQ]), op=ADD)
                    for i in range(DC):
                        nc.vector.tensor_scalar(
                            xT[:, i, :], xT[:, i, :],
                            scalar1=lns[:, li, i:i + 1], scalar2=lnb[:, li, i:i + 1],
                            op0=MUL, op1=ADD)

                layer_norm(0)

                for l in range(L):
                    bnc_in, bnc_out = bnc_ins[l], bnc_outs[l]
                    xbf = act.tile([128, DC, SQ], BF16, tag="xbf")
                    nc.vector.tensor_copy(xbf[:], xT[:])
                    wk = wp.tile([128, DC, D], BF16, tag="wk")
                    nc.sync.dma_start(wk[:], WK[l])
                    wv = wp.tile([128, DC, D], BF16, tag="wv")
                    nc.sync.dma_start(wv[:], WV[l])
                    wq = wp.tile([128, DC, D], BF16, tag="wq")
                    nc.sync.dma_start(wq[:], WQ[l])
                    bq = wp.tile([128, DC], F32, tag="bq")
                    nc.sync.dma_start(bq[:], BQ[l])
                    bk = wp.tile([128, DC], F32, tag="bk")
                    nc.sync.dma_start(bk[:], BK[l])
                    bvr = wp.tile([1, D], BF16, tag="bvr")
                    nc.sync.dma_start(bvr[:], BROW[l][0:1, :])
                    bor = wp.tile([1, D], BF16, tag="bor")
                    nc.sync.dma_start(bor[:], BROW[l][1:2, :])
                    b2r = wp.tile([1, D], BF16, tag="b2r")
                    nc.sync.dma_start(b2r[:], BROW[l][2:3, :])

                    # K^T shard
                    ktb = act.tile([128, DC, SQ], BF16, tag="ktb", bufs=1)
                    for oc in range(DC):
                        kp = ps.tile([128, SQ], F32, tag="proj")
                        for dc in range(DC):
                            nc.tensor.matmul(kp[:], wk[:, dc, 128 * oc:128 * (oc + 1)],
                                             xbf[:, dc, :], start=(dc == 0),
                                             stop=(dc == DC - 1))
                        nc.vector.tensor_scalar_add(ktb[:, oc, :], kp[:],
                                                    bk[:, oc:oc + 1])
                    # V normal shard [128, 2, H, 65] in two 384-wide halves
                    vnb = act.tile([128, 2, H, VN_E], BF16, tag="vnb", bufs=1)
                    nc.vector.memset(vnb[:, :, :, HD:], 1.0)
                    for rc in range(2):
                        for hf in range(2):
                            cs = slice(384 * hf, 384 * (hf + 1))
                            vp = ps.tile([128, 384], F32, tag="proj", name="vproj")
                            for dc in range(DC):
                                nc.tensor.matmul(
                                    vp[:], xbf[:, dc, 128 * rc:128 * (rc + 1)],
                                    wv[:, dc, cs], start=(dc == 0), stop=False)
                            nc.tensor.matmul(vp[:], onesrow[:1, :128],
                                             bvr[:1, cs], start=False, stop=True)
                            nc.vector.tensor_copy(
                                vnb[:, rc, 6 * hf:6 * (hf + 1), 0:HD],
                                vp[:].rearrange("p (h e) -> p h e", h=6))
                    nc.gpsimd.dma_start(
                        bnc_in[0:VN_FLAT].rearrange(
                            "(rc p h e) -> p rc h e", rc=2, p=128, h=H),
                        vnb[:])
                    nc.gpsimd.dma_start(
                        bnc_in[VN_FLAT:AG_FLAT].rearrange(
                            "(i p q) -> p i q", i=DC, p=128),
                        ktb[:])
                    nc.gpsimd.collective_compute(
                        "AllGather", mybir.AluOpType.bypass,
                        replica_groups=[list(range(NC))],
                        ins=[bnc_in[:].opt()], outs=[bnc_out[:].opt()])

                    # Q^T while AG flies
                    qtb = act.tile([128, DC, SQ], BF16, tag="qtb", bufs=1)
                    for oc in range(DC):
                        qp = ps.tile([128, SQ], F32, tag="proj")
                        for dc in range(DC):
                            nc.tensor.matmul(qp[:], wq[:, dc, 128 * oc:128 * (oc + 1)],
                                             xbf[:, dc, :], start=(dc == 0),
                                             stop=(dc == DC - 1))
                        nc.vector.tensor_scalar(qtb[:, oc, :], qp[:],
                                                scalar1=0.125,
                                                scalar2=bq[:, oc:oc + 1],
                                                op0=MUL, op1=ADD)

                    ktag = ag.tile([128, NC * DC, SQ], BF16, tag="ktag")
                    vnag = ag.tile([128, KC, H, VN_E], BF16, tag="vnag")
                    for r in range(NC):
                        o = AG_FLAT * r
                        nc.gpsimd.dma_start(
                            vnag[:, 2 * r:2 * r + 2, :, :],
                            bnc_out[o:o + VN_FLAT].rearrange(
                                "(rc p h e) -> p rc h e", rc=2, p=128, h=H))
                        nc.gpsimd.dma_start(
                            ktag[:, DC * r:DC * (r + 1), :],
                            bnc_out[o + VN_FLAT:o + AG_FLAT].rearrange(
                                "(i p q) -> p i q", i=DC, p=128))

                    wo = wp.tile([128, DC, D], BF16, tag="wo")
                    nc.sync.dma_start(wo[:], WO[l])

                    # attention; ctx raw + per-head recip rows
                    ctxr = act.tile([128, DC, SQ], F32, tag="ctxr", bufs=1)
                    ctxb = act.tile([128, DC, SQ], BF16, tag="ctxb", bufs=1)
                    for h in range(H):
                        ki, kp_ = h // 2, 64 * (h % 2)
                        ct = ps1.tile([VN_E, SQ], F32, tag="ctx")
                        for w in range(4):
                            sc = pssc.tile([128, 4, SQ], F32, tag="sc")
                            for j in range(4):
                                kc = 4 * w + j
                                r, hf = kc // 2, kc % 2
                                nc.tensor.matmul(
                                    sc[:, j, :],
                                    ktag[kp_:kp_ + 64, DC * r + ki,
                                         128 * hf:128 * (hf + 1)],
                                    qtb[kp_:kp_ + 64, ki, :],
                                    start=True, stop=True)
                            pr = sm.tile([128, 4, SQ], BF16, tag="pr")
                            nc.scalar.activation(pr[:], sc[:], AF.Exp)
                            nc.vector.tensor_tensor(
                                pr[:].rearrange("p j (b q) -> p j b q", b=4),
                                pr[:].rearrange("p j (b q) -> p j b q", b=4),
                                m01[:, 4 * w:4 * w + 4, :, None].to_broadcast(
                                    [128, 4, 4, 64]),
                                op=MUL)
                            for j in range(4):
                                kc = 4 * w + j
                                nc.tensor.matmul(
                                    ct[:], vnag[:, kc, h, :], pr[:, j, :],
                                    start=(kc == 0), stop=(kc == KC - 1))
                        rech = sm.tile([1, SQ], F32, tag="rech")
                        nc.vector.reciprocal(rech[:], ct[HD:HD + 1, :])
                        nc.vector.tensor_copy(ctxr[kp_:kp_ + 64, ki, :], ct[0:HD, :])
                        rbc = pssc.tile([64, SQ], F32, tag="sc", name="rbc")
                        nc.tensor.matmul(rbc[:], ones_fr[:1, :64], rech[:],
                                         start=True, stop=True)
                        nc.vector.tensor_tensor(ctxb[kp_:kp_ + 64, ki, :],
                                                ctxr[kp_:kp_ + 64, ki, :],
                                                rbc[:], op=MUL)

                    # Wo + residual
                    for oc in range(DC):
                        op_ = ps.tile([128, SQ], F32, tag="proj")
                        for dc in range(DC):
                            nc.tensor.matmul(op_[:], wo[:, dc, 128 * oc:128 * (oc + 1)],
                                             ctxb[:, dc, :], start=(dc == 0),
                                             stop=False)
                        nc.tensor.matmul(op_[:], bor[:1, 128 * oc:128 * (oc + 1)],
                                         onesrow[:1, :SQ], start=False, stop=True)
                        nc.vector.tensor_tensor(xT[:, oc, :], xT[:, oc, :], op_[:],
                                                op=ADD)
                    layer_norm(1 + 2 * l)

                    # FFN (W1/W2 streamed in column halves to fit SBUF)
                    b1 = wp.tile([128, FC], F32, tag="b1")
                    nc.sync.dma_start(b1[:], B1[l])
                    xbf2 = act.tile([128, DC, SQ], BF16, tag="xbf", name="xbf2")
                    nc.vector.tensor_copy(xbf2[:], xT[:])
                    htb = act.tile([128, FC, SQ], BF16, tag="htb", bufs=1)
                    for w1h_i in range(2):
                        w1h = wp.tile([128, DC, F // 2], BF16, tag="w1h")
                        nc.sync.dma_start(
                            w1h[:], W1[l][:, :, (F // 2) * w1h_i:(F // 2) * (w1h_i + 1)])
                        for oc12 in range(12):
                            oc = 12 * w1h_i + oc12
                            hp = ps.tile([128, SQ], F32, tag="proj")
                            for dc in range(DC):
                                nc.tensor.matmul(
                                    hp[:], w1h[:, dc, 128 * oc12:128 * (oc12 + 1)],
                                    xbf2[:, dc, :], start=(dc == 0),
                                    stop=(dc == DC - 1))
                            nc.scalar.activation(htb[:, oc, :], hp[:],
                                                 AF.Gelu_apprx_tanh,
                                                 bias=b1[:, oc:oc + 1])
                    for w2h_i in range(2):
                        w2h = wp.tile([128, FC, D // 2], BF16, tag="w2h")
                        nc.sync.dma_start(
                            w2h[:], W2[l][:, :, (D // 2) * w2h_i:(D // 2) * (w2h_i + 1)])
                        for oc3 in range(3):
                            oc = 3 * w2h_i + oc3
                            yp = ps.tile([128, SQ], F32, tag="proj")
                            for kc in range(FC):
                                nc.tensor.matmul(
                                    yp[:], w2h[:, kc, 128 * oc3:128 * (oc3 + 1)],
                                    htb[:, kc, :], start=(kc == 0), stop=False)
                            nc.tensor.matmul(yp[:],
                                             b2r[:1, 128 * oc:128 * (oc + 1)],
                                             onesrow[:1, :SQ], start=False, stop=True)
                            nc.vector.tensor_tensor(xT[:, oc, :], xT[:, oc, :], yp[:],
                                                    op=ADD)
                    layer_norm(2 + 2 * l)

            with tc.tile_pool(name="fin", bufs=1) as fin, \
                 tc.tile_pool(name="fw", bufs=4) as fw, \
                 tc.tile_pool(name="fo", bufs=3) as fo, \
                 tc.tile_pool(name="fps", bufs=2, space="PSUM") as fps:
                xbf = fin.tile([128, DC, SQ], BF16)
                nc.vector.tensor_copy(xbf[:], xT[:])
                nc.gpsimd.dma_start(
                    bncx_in[:].rearrange("(i p q) -> p i q", i=DC, p=128), xbf[:])
                nc.gpsimd.collective_compute(
                    "AllGather", mybir.AluOpType.bypass,
                    replica_groups=[list(range(NC))],
                    ins=[bncx_in[:].opt()], outs=[bncx_out[:].opt()])
                xf = fin.tile([128, DC, S], BF16)
                for r in range(NC):
                    nc.gpsimd.dma_start(
                        xf[:, :, SQ * r:SQ * (r + 1)],
                        bncx_out[XF_FLAT * r:XF_FLAT * (r + 1)].rearrange(
                            "(i p q) -> p i q", i=DC, p=128))
                fcb = fin.tile([1, VSH], BF16)
                nc.sync.dma_start(fcb[:], FCB[:])
                for oc in range(VSH // 128):
                    fwt = fw.tile([128, DC, 128], BF16, tag="fwt")
                    nc.sync.dma_start(fwt[:], FCW[:, :, 128 * oc:128 * (oc + 1)])
                    fp = fps.tile([128, S], F32, tag="fp")
                    for ncol in range(4):
                        cs = slice(512 * ncol, 512 * (ncol + 1))
                        for dc in range(DC):
                            nc.tensor.matmul(fp[:, cs], fwt[:, dc, :],
                                             xf[:, dc, cs], start=(dc == 0),
                                             stop=False)
                        nc.tensor.matmul(fp[:, cs],
                                         fcb[0:1, 128 * oc:128 * (oc + 1)],
                                         onesrow[:1, :], start=False, stop=True)
                    ot = fo.tile([128, S], F32, tag="ot")
                    if oc % 2 == 0:
                        nc.scalar.activation(ot[:], fp[:], AF.Copy)
                    else:
                        nc.vector.tensor_copy(ot[:], fp[:])
                    nc.sync.dma_start(OUT[oc], ot[:])
    nc.finalize()
    return nc


def kernel(input_ids, attention_mask, token_type_ids, word_emb, pos_emb, type_emb,
           emb_ln_s, emb_ln_b, Wq, Wk, Wv, bq, bk, bv, Wo, bo, ln1_s, ln1_b,
           W1, b1, W2, b2, ln2_s, ln2_b, fc_w, fc_b):
    f32 = np.float32
    bf = ml_dtypes.bfloat16
    ids = np.asarray(input_ids)[0]
    tt = np.asarray(token_type_ids)[0]
    am = np.asarray(attention_mask)[0].astype(f32)
    E = (np.asarray(word_emb, f32)[ids] + np.asarray(pos_emb, f32)
         + np.asarray(type_emb, f32)[tt])

    def tp(w):  # [D, N] -> [128, DC, N] bf16
        return np.ascontiguousarray(
            np.asarray(w, f32).reshape(-1, 128, w.shape[-1]).transpose(1, 0, 2)
        ).astype(bf)

    def col(b, scale=1.0):  # [N*128] -> [128, N] f32
        b = np.asarray(b, f32) * scale
        return np.ascontiguousarray(b.reshape(-1, 128).T).astype(f32)

    wq_h = np.stack([tp(Wq[l]) for l in range(L)])
    wk_h = np.stack([tp(Wk[l]) for l in range(L)])
    wv_h = np.stack([tp(Wv[l]) for l in range(L)])
    wo_h = np.stack([tp(Wo[l]) for l in range(L)])
    w1_h = np.stack([tp(W1[l]) for l in range(L)])
    w2_h = np.stack([tp(W2[l]) for l in range(L)])
    bq_h = np.stack([col(bq[l], 0.125) for l in range(L)])
    bk_h = np.stack([col(bk[l]) for l in range(L)])
    b1_h = np.stack([col(b1[l]) for l in range(L)])
    brow_h = np.stack([np.stack([np.asarray(bv[l], f32), np.asarray(bo[l], f32),
                                 np.asarray(b2[l], f32)]) for l in range(L)]).astype(bf)

    lns_list = [np.asarray(emb_ln_s, f32)]
    lnb_list = [np.asarray(emb_ln_b, f32)]
    for l in range(L):
        lns_list += [np.asarray(ln1_s[l], f32), np.asarray(ln2_s[l], f32)]
        lnb_list += [np.asarray(ln1_b[l], f32), np.asarray(ln2_b[l], f32)]
    ln_s = np.stack([s.reshape(DC, 128).T for s in lns_list], axis=1)
    ln_b = np.stack([s.reshape(DC, 128).T for s in lnb_list], axis=1)

    blk = _block_map(NB, R)
    allowed = np.zeros((NB, NB), f32)
    for i in range(NB):
        allowed[i, blk[i]] = 1.0
    allowed[0, :] = 1.0
    allowed[NB - 1, :] = 1.0
    allow_key = np.repeat(allowed, BS, axis=1) * am[None, :]

    fcw_pad = np.zeros((D, VSH * NC), f32)
    fcw_pad[:, :V] = np.asarray(fc_w, f32)
    fcb_pad = np.zeros((VSH * NC,), f32)
    fcb_pad[:V] = np.asarray(fc_b, f32)

    if "k" not in _nc_cache:
        _nc_cache["k"] = build()
    nc = _nc_cache["k"]

    in_maps = []
    for c in range(NC):
        e_shard = E[SQ * c:SQ * (c + 1)].T.reshape(DC, 128, SQ).transpose(1, 0, 2)
        m01 = allow_key[4 * c:4 * c + 4].T.reshape(KC, 128, 4).transpose(1, 0, 2)
        in_maps.append({
            "e_t": np.ascontiguousarray(e_shard).astype(f32),
            "wq": wq_h, "wk": wk_h, "wv": wv_h, "wo": wo_h,
            "w1": w1_h, "w2": w2_h,
            "bq_t": bq_h, "bk_t": bk_h, "b1_t": b1_h, "brow": brow_h,
            "ln_s": ln_s, "ln_b": ln_b,
            "m01t": np.ascontiguousarray(m01).astype(bf),
            "fcw": tp(fcw_pad[:, VSH * c:VSH * (c + 1)]),
            "fcb": fcb_pad[None, VSH * c:VSH * (c + 1)].astype(bf),
        })

    trace = bool(int(os.environ.get("BB_TRACE", "0")))
    res = run_bass_kernel_spmd(nc, in_maps, core_ids=list(range(NC)), trace=trace)
    kernel.last_exec_ns = getattr(res, "exec_time_ns", None)
    outs = [res.results[c]["out_t"].reshape(VSH, S).T for c in range(NC)]
    logits = np.concatenate(outs, axis=1)[:, :V]
    return logits[None].astype(np.float32)



# revision 3
# speedup vs baseline: 28039.5249x; 28039.5249x over previous
"""BigBird encoder + vocab projection on 8 Trainium2 NeuronCores.

Sequence-sharded transformer (core c owns rows [256c, 256c+256) = 4 query
blocks), per-layer AllGather of K^T and V, vocab-sharded final projection.
Activations live transposed (xT [768part, 256free]) so every matmul consumes
weights as stored.  BigBird sparsity (window + global + random + dedup +
key_mask) is folded into a per-core 0/1 mask multiplied into exp(scores) —
mathematically identical to the reference's gather+softmax.  bf16 matmuls,
fp32 accumulate/residual.
"""
import os, sys
os.environ.setdefault("JAX_PLATFORMS", "")
import numpy as np
import ml_dtypes

sys.path.insert(0, "/opt/trn_rl_repo")

import concourse.bass as bass
import concourse.tile as tile
from concourse import bacc, mybir
from concourse.bass_utils import run_bass_kernel_spmd

BF16 = mybir.dt.bfloat16
F32 = mybir.dt.float32
AF = mybir.ActivationFunctionType
MUL = mybir.AluOpType.mult
ADD = mybir.AluOpType.add
SUB = mybir.AluOpType.subtract

B, S, D, F, V = 1, 2048, 768, 3072, 50358
L, H, HD, BS, NB, R = 12, 12, 64, 64, 32, 3
NC = 8
SQ = S // NC                # 256
DC = D // 128               # 6
FC = F // 128               # 24
KC = S // 128               # 16
VSH = 6400                  # padded vocab shard (50 x 128)
VN_E = HD + 1               # 65 cols per head in V-normal (ones col for rowsum)
VN_FLAT = 128 * 2 * H * VN_E    # 199680
KT_FLAT = 128 * DC * SQ         # 196608
AG_FLAT = VN_FLAT + KT_FLAT     # 396288
XF_FLAT = 128 * DC * SQ

_nc_cache = {}


def _block_map(nb, r, seed=0):
    rng = np.random.default_rng(seed)
    idx = np.zeros((nb, 5 + r), np.int32)
    for i in range(nb):
        lst = [0, nb - 1, max(i - 1, 0), i, min(i + 1, nb - 1)]
        cand = np.setdiff1d(np.arange(nb), np.array(lst))
        lst += list(rng.choice(cand, r, replace=False))
        for j, b in enumerate(lst):
            idx[i, j] = int(b)
    return idx


def build():
    nc = bacc.Bacc("TRN2", target_bir_lowering=False, debug=False, num_devices=NC)
    ET = nc.dram_tensor("e_t", [128, DC, SQ], F32, kind="ExternalInput")
    WQ = nc.dram_tensor("wq", [L, 128, DC, D], BF16, kind="ExternalInput")
    WK = nc.dram_tensor("wk", [L, 128, DC, D], BF16, kind="ExternalInput")
    WV = nc.dram_tensor("wv", [L, 128, DC, D], BF16, kind="ExternalInput")
    WO = nc.dram_tensor("wo", [L, 128, DC, D], BF16, kind="ExternalInput")
    W1 = nc.dram_tensor("w1", [L, 128, DC, F], BF16, kind="ExternalInput")
    W2 = nc.dram_tensor("w2", [L, 128, FC, D], BF16, kind="ExternalInput")
    BQ = nc.dram_tensor("bq_t", [L, 128, DC], F32, kind="ExternalInput")  # x0.125
    BK = nc.dram_tensor("bk_t", [L, 128, DC], F32, kind="ExternalInput")
    B1 = nc.dram_tensor("b1_t", [L, 128, FC], F32, kind="ExternalInput")
    BROW = nc.dram_tensor("brow", [L, 3, D], BF16, kind="ExternalInput")  # bv,bo,b2
    LNS = nc.dram_tensor("ln_s", [128, 2 * L + 1, DC], F32, kind="ExternalInput")
    LNB = nc.dram_tensor("ln_b", [128, 2 * L + 1, DC], F32, kind="ExternalInput")
    M01 = nc.dram_tensor("m01t", [128, KC, 4], BF16, kind="ExternalInput")
    FCW = nc.dram_tensor("fcw", [128, DC, VSH], BF16, kind="ExternalInput")
    FCB = nc.dram_tensor("fcb", [1, VSH], BF16, kind="ExternalInput")
    OUT = nc.dram_tensor("out_t", [VSH // 128, 128, S], F32, kind="ExternalOutput")

    with tile.TileContext(nc) as tc:
        with tc.tile_pool(name="dram", bufs=1, space="DRAM") as dram, \
             tc.tile_pool(name="res", bufs=1) as res, \
             tc.tile_pool(name="const", bufs=1) as const:
            bnc_ins = [dram.tile([AG_FLAT], BF16, tag=f"bin{l}", name=f"bin{l}")
                       for l in range(L)]
            bnc_outs = [dram.tile([NC * AG_FLAT], BF16, addr_space="Shared",
                                  tag=f"bout{l}", name=f"bout{l}")
                        for l in range(L)]
            bncx_in = dram.tile([XF_FLAT], BF16)
            bncx_out = dram.tile([NC * XF_FLAT], BF16, addr_space="Shared")

            xT = res.tile([128, DC, SQ], F32)
            nc.sync.dma_start(xT[:], ET[:])
            m01 = const.tile([128, KC, 4], BF16)
            nc.sync.dma_start(m01[:], M01[:])
            lns = const.tile([128, 2 * L + 1, DC], F32)
            lnb = const.tile([128, 2 * L + 1, DC], F32)
            nc.sync.dma_start(lns[:], LNS[:])
            nc.sync.dma_start(lnb[:], LNB[:])
            ones_b = const.tile([128, 1], BF16)
            nc.vector.memset(ones_b[:], 1.0)
            ones_fr = const.tile([1, 128], F32)
            nc.vector.memset(ones_fr[:], 1.0)
            onesrow = const.tile([1, 512], BF16)
            nc.vector.memset(onesrow[:], 1.0)
            eps = const.tile([1, 1], F32)
            nc.vector.memset(eps[:], 1e-12)

            with tc.tile_pool(name="wp", bufs=1) as wp, \
                 tc.tile_pool(name="act", bufs=2) as act, \
                 tc.tile_pool(name="ag", bufs=1) as ag, \
                 tc.tile_pool(name="ps", bufs=2, space="PSUM") as ps, \
                 tc.tile_pool(name="pssc", bufs=2, space="PSUM") as pssc, \
                 tc.tile_pool(name="ps1", bufs=2, space="PSUM") as ps1, \
                 tc.tile_pool(name="sm", bufs=2) as sm:

                def layer_norm(li):
                    xbf = act.tile([128, DC, SQ], BF16, tag="xbf", name="lnxbf")
                    nc.vector.tensor_copy(xbf[:], xT[:])
                    sq = act.tile([128, DC, SQ], BF16, tag="sq", bufs=1)
                    nc.scalar.activation(sq[:], xbf[:], AF.Square)
                    sum_ps = pssc.tile([1, SQ], F32, tag="sc", name="lnsum")
                    ssq_ps = pssc.tile([1, SQ], F32, tag="sc", name="lnssq")
                    for i in range(DC):
                        nc.tensor.matmul(sum_ps[:], ones_b[:], xbf[:, i, :],
                                         start=(i == 0), stop=(i == DC - 1))
                    for i in range(DC):
                        nc.tensor.matmul(ssq_ps[:], ones_b[:], sq[:, i, :],
                                         start=(i == 0), stop=(i == DC - 1))
                    nmean = sm.tile([1, SQ], F32, tag="nmean")
                    ms = sm.tile([1, SQ], F32, tag="ms")
                    nc.vector.tensor_scalar_mul(nmean[:], sum_ps[:], -1.0 / D)
                    nc.vector.tensor_scalar_mul(ms[:], ssq_ps[:], 1.0 / D)
                    ab = sm.tile([1, 2 * SQ], F32, tag="ab")
                    # var -> ab[0:SQ]
                    nc.vector.tensor_tensor(ab[:, 0:SQ], nmean[:], nmean[:], op=MUL)
                    nc.vector.tensor_tensor(ab[:, 0:SQ], ms[:], ab[:, 0:SQ], op=SUB)
                    nc.scalar.activation(ab[:, 0:SQ], ab[:, 0:SQ], AF.Sqrt,
                                         bias=eps[:])
                    nc.vector.reciprocal(ab[:, 0:SQ], ab[:, 0:SQ])   # rstd
                    nc.vector.tensor_tensor(ab[:, SQ:], nmean[:], ab[:, 0:SQ],
                                            op=MUL)                  # -mean*rstd
                    bc = pssc.tile([128, 2 * SQ], F32, tag="sc", name="lnbc")
                    nc.tensor.matmul(bc[:], ones_fr[:], ab[:], start=True, stop=True)
                    nc.vector.tensor_tensor(
                        xT[:], xT[:],
                        bc[:, None, 0:SQ].to_broadcast([128, DC, SQ]), op=MUL)
                    nc.vector.tensor_tensor(
                        xT[:], xT[:],
                        bc[:, None, SQ:2 * SQ].to_broadcast([128, DC, SQ]), op=ADD)
                    for i in range(DC):
                        nc.vector.tensor_scalar(
                            xT[:, i, :], xT[:, i, :],
                            scalar1=lns[:, li, i:i + 1], scalar2=lnb[:, li, i:i + 1],
                            op0=MUL, op1=ADD)

                layer_norm(0)

                for l in range(L):
                    bnc_in, bnc_out = bnc_ins[l], bnc_outs[l]
                    xbf = act.tile([128, DC, SQ], BF16, tag="xbf")
                    nc.vector.tensor_copy(xbf[:], xT[:])
                    wk = wp.tile([128, DC, D], BF16, tag="wk")
                    nc.sync.dma_start(wk[:], WK[l])
                    wv = wp.tile([128, DC, D], BF16, tag="wv")
                    nc.sync.dma_start(wv[:], WV[l])
                    wq = wp.tile([128, DC, D], BF16, tag="wq")
                    nc.sync.dma_start(wq[:], WQ[l])
                    bq = wp.tile([128, DC], F32, tag="bq")
                    nc.sync.dma_start(bq[:], BQ[l])
                    bk = wp.tile([128, DC], F32, tag="bk")
                    nc.sync.dma_start(bk[:], BK[l])
                    bvr = wp.tile([1, D], BF16, tag="bvr")
                    nc.sync.dma_start(bvr[:], BROW[l][0:1, :])
                    bor = wp.tile([1, D], BF16, tag="bor")
                    nc.sync.dma_start(bor[:], BROW[l][1:2, :])
                    b2r = wp.tile([1, D], BF16, tag="b2r")
                    nc.sync.dma_start(b2r[:], BROW[l][2:3, :])

                    # K^T shard
                    ktb = act.tile([128, DC, SQ], BF16, tag="ktb", bufs=1)
                    for oc in range(DC):
                        kp = ps.tile([128, SQ], F32, tag="proj")
                        for dc in range(DC):
                            nc.tensor.matmul(kp[:], wk[:, dc, 128 * oc:128 * (oc + 1)],
                                             xbf[:, dc, :], start=(dc == 0),
                                             stop=(dc == DC - 1))
                        nc.vector.tensor_scalar_add(ktb[:, oc, :], kp[:],
                                                    bk[:, oc:oc + 1])
                    # V normal shard [128, 2, H, 65] in two 384-wide halves
                    vnb = act.tile([128, 2, H, VN_E], BF16, tag="vnb", bufs=1)
                    nc.vector.memset(vnb[:, :, :, HD:], 1.0)
                    for rc in range(2):
                        for hf in range(2):
                            cs = slice(384 * hf, 384 * (hf + 1))
                            vp = ps.tile([128, 384], F32, tag="proj", name="vproj")
                            for dc in range(DC):
                                nc.tensor.matmul(
                                    vp[:], xbf[:, dc, 128 * rc:128 * (rc + 1)],
                                    wv[:, dc, cs], start=(dc == 0), stop=False)
                            nc.tensor.matmul(vp[:], onesrow[:1, :128],
                                             bvr[:1, cs], start=False, stop=True)
                            nc.vector.tensor_copy(
                                vnb[:, rc, 6 * hf:6 * (hf + 1), 0:HD],
                                vp[:].rearrange("p (h e) -> p h e", h=6))
                    nc.gpsimd.dma_start(
                        bnc_in[0:VN_FLAT].rearrange(
                            "(rc p h e) -> p rc h e", rc=2, p=128, h=H),
                        vnb[:])
                    nc.gpsimd.dma_start(
                        bnc_in[VN_FLAT:AG_FLAT].rearrange(
                            "(i p q) -> p i q", i=DC, p=128),
                        ktb[:])
                    nc.gpsimd.collective_compute(
                        "AllGather", mybir.AluOpType.bypass,
                        replica_groups=[list(range(NC))],
                        ins=[bnc_in[:].opt()], outs=[bnc_out[:].opt()])

                    # Q^T while AG flies
                    qtb = act.tile([128, DC, SQ], BF16, tag="qtb", bufs=1)
                    for oc in range(DC):
                        qp = ps.tile([128, SQ], F32, tag="proj")
                        for dc in range(DC):
                            nc.tensor.matmul(qp[:], wq[:, dc, 128 * oc:128 * (oc + 1)],
                                             xbf[:, dc, :], start=(dc == 0),
                                             stop=(dc == DC - 1))
                        nc.vector.tensor_scalar(qtb[:, oc, :], qp[:],
                                                scalar1=0.125,
                                                scalar2=bq[:, oc:oc + 1],
                                                op0=MUL, op1=ADD)

                    ktag = ag.tile([128, NC * DC, SQ], BF16, tag="ktag")
                    vnag = ag.tile([128, KC, H, VN_E], BF16, tag="vnag")
                    for r in range(NC):
                        o = AG_FLAT * r
                        nc.gpsimd.dma_start(
                            vnag[:, 2 * r:2 * r + 2, :, :],
                            bnc_out[o:o + VN_FLAT].rearrange(
                                "(rc p h e) -> p rc h e", rc=2, p=128, h=H))
                        nc.gpsimd.dma_start(
                            ktag[:, DC * r:DC * (r + 1), :],
                            bnc_out[o + VN_FLAT:o + AG_FLAT].rearrange(
                                "(i p q) -> p i q", i=DC, p=128))

                    wo = wp.tile([128, DC, D], BF16, tag="wo")
                    nc.sync.dma_start(wo[:], WO[l])

                    # attention; ctx raw + per-head recip rows
                    ctxr = act.tile([128, DC, SQ], F32, tag="ctxr", bufs=1)
                    ctxb = act.tile([128, DC, SQ], BF16, tag="ctxb", bufs=1)
                    for h in range(H):
                        ki, kp_ = h // 2, 64 * (h % 2)
                        ct = ps1.tile([VN_E, SQ], F32, tag="ctx")
                        for w in range(4):
                            sc = pssc.tile([128, 4, SQ], F32, tag="sc")
                            for j in range(4):
                                kc = 4 * w + j
                                r, hf = kc // 2, kc % 2
                                nc.tensor.matmul(
                                    sc[:, j, :],
                                    ktag[kp_:kp_ + 64, DC * r + ki,
                                         128 * hf:128 * (hf + 1)],
                                    qtb[kp_:kp_ + 64, ki, :],
                                    start=True, stop=True)
                            pr = sm.tile([128, 4, SQ], BF16, tag="pr")
                            nc.scalar.activation(pr[:], sc[:], AF.Exp)
                            nc.vector.tensor_tensor(
                                pr[:].rearrange("p j (b q) -> p j b q", b=4),
                                pr[:].rearrange("p j (b q) -> p j b q", b=4),
                                m01[:, 4 * w:4 * w + 4, :, None].to_broadcast(
                                    [128, 4, 4, 64]),
                                op=MUL)
                            for j in range(4):
                                kc = 4 * w + j
                                nc.tensor.matmul(
                                    ct[:], vnag[:, kc, h, :], pr[:, j, :],
                                    start=(kc == 0), stop=(kc == KC - 1))
                        rech = sm.tile([1, SQ], F32, tag="rech")
                        nc.vector.reciprocal(rech[:], ct[HD:HD + 1, :])
                        nc.vector.tensor_copy(ctxr[kp_:kp_ + 64, ki, :], ct[0:HD, :])
                        rbc = pssc.tile([64, SQ], F32, tag="sc", name="rbc")
                        nc.tensor.matmul(rbc[:], ones_fr[:1, :64], rech[:],
                                         start=True, stop=True)
                        nc.vector.tensor_tensor(ctxb[kp_:kp_ + 64, ki, :],
                                                ctxr[kp_:kp_ + 64, ki, :],
                                                rbc[:], op=MUL)

                    # Wo + residual
                    for oc in range(DC):
                        op_ = ps.tile([128, SQ], F32, tag="proj")
                        for dc in range(DC):
                            nc.tensor.matmul(op_[:], wo[:, dc, 128 * oc:128 * (oc + 1)],
                                             ctxb[:, dc, :], start=(dc == 0),
                                             stop=False)
                        nc.tensor.matmul(op_[:], bor[:1, 128 * oc:128 * (oc + 1)],
                                         onesrow[:1, :SQ], start=False, stop=True)
                        nc.vector.tensor_tensor(xT[:, oc, :], xT[:, oc, :], op_[:],
                                                op=ADD)
                    layer_norm(1 + 2 * l)

                    # FFN (W1/W2 streamed in column halves to fit SBUF)
                    b1 = wp.tile([128, FC], F32, tag="b1")
                    nc.sync.dma_start(b1[:], B1[l])
                    xbf2 = act.tile([128, DC, SQ], BF16, tag="xbf", name="xbf2")
                    nc.vector.tensor_copy(xbf2[:], xT[:])
                    htb = act.tile([128, FC, SQ], BF16, tag="htb", bufs=1)
                    for w1h_i in range(2):
                        w1h = wp.tile([128, DC, F // 2], BF16, tag="w1h")
                        nc.sync.dma_start(
                            w1h[:], W1[l][:, :, (F // 2) * w1h_i:(F // 2) * (w1h_i + 1)])
                        for oc12 in range(12):
                            oc = 12 * w1h_i + oc12
                            hp = ps.tile([128, SQ], F32, tag="proj")
                            for dc in range(DC):
                                nc.tensor.matmul(
                                    hp[:], w1h[:, dc, 128 * oc12:128 * (oc12 + 1)],
                                    xbf2[:, dc, :], start=(dc == 0),
                                    stop=(dc == DC - 1))
                            nc.scalar.activation(htb[:, oc, :], hp[:],
                                                 AF.Gelu_apprx_tanh,
                                                 bias=b1[:, oc:oc + 1])
                    for w2h_i in range(2):
                        w2h = wp.tile([128, FC, D // 2], BF16, tag="w2h")
                        nc.sync.dma_start(
                            w2h[:], W2[l][:, :, (D // 2) * w2h_i:(D // 2) * (w2h_i + 1)])
                        for oc3 in range(3):
                            oc = 3 * w2h_i + oc3
                            yp = ps.tile([128, SQ], F32, tag="proj")
                            for kc in range(FC):
                                nc.tensor.matmul(
                                    yp[:], w2h[:, kc, 128 * oc3:128 * (oc3 + 1)],
                                    htb[:, kc, :], start=(kc == 0), stop=False)
                            nc.tensor.matmul(yp[:],
                                             b2r[:1, 128 * oc:128 * (oc + 1)],
                                             onesrow[:1, :SQ], start=False, stop=True)
                            nc.vector.tensor_tensor(xT[:, oc, :], xT[:, oc, :], yp[:],
                                                    op=ADD)
                    layer_norm(2 + 2 * l)

            with tc.tile_pool(name="fin", bufs=1) as fin, \
                 tc.tile_pool(name="fw", bufs=4) as fw, \
                 tc.tile_pool(name="fo", bufs=3) as fo, \
                 tc.tile_pool(name="fps", bufs=2, space="PSUM") as fps:
                xbf = fin.tile([128, DC, SQ], BF16)
                nc.vector.tensor_copy(xbf[:], xT[:])
                nc.gpsimd.dma_start(
                    bncx_in[:].rearrange("(i p q) -> p i q", i=DC, p=128), xbf[:])
                nc.gpsimd.collective_compute(
                    "AllGather", mybir.AluOpType.bypass,
                    replica_groups=[list(range(NC))],
                    ins=[bncx_in[:].opt()], outs=[bncx_out[:].opt()])
                xf = fin.tile([128, DC, S], BF16)
                for r in range(NC):
                    nc.gpsimd.dma_start(
                        xf[:, :, SQ * r:SQ * (r + 1)],
                        bncx_out[XF_FLAT * r:XF_FLAT * (r + 1)].rearrange(
                            "(i p q) -> p i q", i=DC, p=128))
                fcb = fin.tile([1, VSH], BF16)
                nc.sync.dma_start(fcb[:], FCB[:])
                for oc in range(VSH // 128):
                    fwt = fw.tile([128, DC, 128], BF16, tag="fwt")
                    nc.sync.dma_start(fwt[:], FCW[:, :, 128 * oc:128 * (oc + 1)])
                    fp = fps.tile([128, S], F32, tag="fp")
                    for ncol in range(4):
                        cs = slice(512 * ncol, 512 * (ncol + 1))
                        for dc in range(DC):
                            nc.tensor.matmul(fp[:, cs], fwt[:, dc, :],
                                             xf[:, dc, cs], start=(dc == 0),
                                             stop=False)
                        nc.tensor.matmul(fp[:, cs],
                                         fcb[0:1, 128 * oc:128 * (oc + 1)],
                                         onesrow[:1, :], start=False, stop=True)
                    ot = fo.tile([128, S], F32, tag="ot")
                    if oc % 2 == 0:
                        nc.scalar.activation(ot[:], fp[:], AF.Copy)
                    else:
                        nc.vector.tensor_copy(ot[:], fp[:])
                    nc.sync.dma_start(OUT[oc], ot[:])
    nc.finalize()
    return nc


def kernel(input_ids, attention_mask, token_type_ids, word_emb, pos_emb, type_emb,
           emb_ln_s, emb_ln_b, Wq, Wk, Wv, bq, bk, bv, Wo, bo, ln1_s, ln1_b,
           W1, b1, W2, b2, ln2_s, ln2_b, fc_w, fc_b):
    f32 = np.float32
    bf = ml_dtypes.bfloat16
    ids = np.asarray(input_ids)[0]
    tt = np.asarray(token_type_ids)[0]
    am = np.asarray(attention_mask)[0].astype(f32)
    E = (np.asarray(word_emb, f32)[ids] + np.asarray(pos_emb, f32)
         + np.asarray(type_emb, f32)[tt])

    def tp(w):  # [D, N] -> [128, DC, N] bf16
        return np.ascontiguousarray(
            np.asarray(w, f32).reshape(-1, 128, w.shape[-1]).transpose(1, 0, 2)
        ).astype(bf)

    def col(b, scale=1.0):  # [N*128] -> [128, N] f32
        b = np.asarray(b, f32) * scale
        return np.ascontiguousarray(b.reshape(-1, 128).T).astype(f32)

    wq_h = np.stack([tp(Wq[l]) for l in range(L)])
    wk_h = np.stack([tp(Wk[l]) for l in range(L)])
    wv_h = np.stack([tp(Wv[l]) for l in range(L)])
    wo_h = np.stack([tp(Wo[l]) for l in range(L)])
    w1_h = np.stack([tp(W1[l]) for l in range(L)])
    w2_h = np.stack([tp(W2[l]) for l in range(L)])
    bq_h = np.stack([col(bq[l], 0.125) for l in range(L)])
    bk_h = np.stack([col(bk[l]) for l in range(L)])
    b1_h = np.stack([col(b1[l]) for l in range(L)])
    brow_h = np.stack([np.stack([np.asarray(bv[l], f32), np.asarray(bo[l], f32),
                                 np.asarray(b2[l], f32)]) for l in range(L)]).astype(bf)

    lns_list = [np.asarray(emb_ln_s, f32)]
    lnb_list = [np.asarray(emb_ln_b, f32)]
    for l in range(L):
        lns_list += [np.asarray(ln1_s[l], f32), np.asarray(ln2_s[l], f32)]
        lnb_list += [np.asarray(ln1_b[l], f32), np.asarray(ln2_b[l], f32)]
    ln_s = np.stack([s.reshape(DC, 128).T for s in lns_list], axis=1)
    ln_b = np.stack([s.reshape(DC, 128).T for s in lnb_list], axis=1)

    blk = _block_map(NB, R)
    allowed = np.zeros((NB, NB), f32)
    for i in range(NB):
        allowed[i, blk[i]] = 1.0
    allowed[0, :] = 1.0
    allowed[NB - 1, :] = 1.0
    allow_key = np.repeat(allowed, BS, axis=1) * am[None, :]

    fcw_pad = np.zeros((D, VSH * NC), f32)
    fcw_pad[:, :V] = np.asarray(fc_w, f32)
    fcb_pad = np.zeros((VSH * NC,), f32)
    fcb_pad[:V] = np.asarray(fc_b, f32)

    if "k" not in _nc_cache:
        _nc_cache["k"] = build()
    nc = _nc_cache["k"]

    in_maps = []
    for c in range(NC):
        e_shard = E[SQ * c:SQ * (c + 1)].T.reshape(DC, 128, SQ).transpose(1, 0, 2)
        m01 = allow_key[4 * c:4 * c + 4].T.reshape(KC, 128, 4).transpose(1, 0, 2)
        in_maps.append({
            "e_t": np.ascontiguousarray(e_shard).astype(f32),
            "wq": wq_h, "wk": wk_h, "wv": wv_h, "wo": wo_h,
            "w1": w1_h, "w2": w2_h,
            "bq_t": bq_h, "bk_t": bk_h, "b1_t": b1_h, "brow": brow_h,
            "ln_s": ln_s, "ln_b": ln_b,
            "m01t": np.ascontiguousarray(m01).astype(bf),
            "fcw": tp(fcw_pad[:, VSH * c:VSH * (c + 1)]),
            "fcb": fcb_pad[None, VSH * c:VSH * (c + 1)].astype(bf),
        })

    trace = bool(int(os.environ.get("BB_TRACE", "0")))
    if trace:
        try:
            import antenv.axon_hooks  # noqa: F401
        except ImportError:
            try:
                import types, antenv
                from trn_agent_boot.trn_boot import _ntff_profile_via_ctypes
                mod = types.ModuleType("antenv.axon_hooks")
                _h = {"h": _ntff_profile_via_ctypes("/opt/axon/libaxon_pjrt.so")}
                mod.get_axon_ntff_profile_hook = lambda: _h["h"]
                mod.set_axon_ntff_profile_hook = lambda v: _h.update(h=v)
                sys.modules["antenv.axon_hooks"] = mod
                antenv.axon_hooks = mod
            except Exception:
                trace = False
    tdir = os.environ.get("BB_TMPDIR") or None
    if tdir:
        os.makedirs(tdir, exist_ok=True)
    res = run_bass_kernel_spmd(nc, in_maps, core_ids=list(range(NC)), trace=trace,
                               tmpdir=tdir)
    kernel.last_exec_ns = getattr(res, "exec_time_ns", None)
    outs = [res.results[c]["out_t"].reshape(VSH, S).T for c in range(NC)]
    logits = np.concatenate(outs, axis=1)[:, :V]
    return logits[None].astype(np.float32)

